# revision 11
# baseline (speedup 1.0000x reference)
"""Trainium2 Bass kernel for nn_EvoBinarizedLayer.

Math: out[p,b,o] = sum_i x[p,b,i]*w[0,p,i,o] + (1-x[p,b,i])*w[1,p,i,o]
                 = (x @ D)[p,b,o] + colsum(W1)[p,o],   D = W0 - W1

All inputs are {0,1}-valued f32, so D is {-1,0,1} and both x and D are
exactly representable in bf16; PSUM accumulates in fp32, so the bf16
matmul path is bit-exact. colsum(W1) enters each PSUM accumulation
group as one extra matmul: ones[128,128].T @ s_bf, where s_bf[k,o] =
sum_{it} w1[it*128+k, o] (values <= 8, bf16-exact) is accumulated on DVE.

Sharding: population dim P=32 split across 8 cores (4 each), no
cross-core communication.

DMA routing: weight loads on the SP HWDGE ring, x loads on the gpsimd
SWDGE queue, output stores on the ACT HWDGE ring - three independent
rings so store-completion stalls never block loads.
"""

import numpy as np

P, B, I, O = 32, 512, 1024, 1024
NCORES = 8
PPC = P // NCORES  # populations per core
NIT = I // 128     # i-tiles (contraction)
NBT = B // 128     # b-tiles
NOH = O // 512     # o-halves (PSUM bank width)

_cache = {}

MAX_WAITS_PER_INST = 1


def _patch_tile_drain():
    """This container's walrus caps sem-waits per TPB_CTRL instruction below
    what Tile's final drain needs; spread the waits across nop instructions."""
    import concourse.tile as tile
    import bass_rust
    from concourse.vector_clock import ScopedClock

    if getattr(tile.TileContext, "_drain_patched", False):
        return

    def _drain_and_barrier(self, tick_clock, wait_clock):
        nc = self.nc
        drain_inst = nc.sync.drain()
        wait_clock.add_sem_waits(
            drain_inst.ins, ScopedClock({None: tick_clock.global_clock})
        )
        si = drain_inst.ins.sync_info
        waits = list(si.on_wait or [])
        if len(waits) > 1:
            si.on_wait = waits[:1]
            drain_inst.ins.sync_info = si
            for i in range(1, len(waits)):
                nop = nc.sync.nop()
                nop.ins.sync_info = bass_rust.SyncInfo(
                    on_wait=[waits[i]], on_update=[]
                )
        nc.all_engine_barrier()
        assert self.sems is not None
        popped = nc._tile_sem_poison_stack.pop()
        assert popped is self._sem_poison
        nc.clear_and_free_semaphores(list(self.sems.allocated().values()))
        nc.all_engine_barrier()

    tile.TileContext._drain_and_barrier = _drain_and_barrier
    tile.TileContext._drain_patched = True


def _split_excess_waits(nc):
    """This container's walrus rejects instructions carrying more than a
    couple of sem-waits; hoist excess waits onto same-engine nops placed
    just before the instruction."""
    import concourse.mybir as mybir
    import bass_rust

    n_split = 0
    for fn in nc.m.functions:
        for bb in fn.blocks:
            new_insts = []
            for inst in bb.instructions:
                si = inst.sync_info
                waits = list(si.on_wait) if si and si.on_wait else []
                if len(waits) > MAX_WAITS_PER_INST:
                    n_split += 1
                    extra = waits[: -MAX_WAITS_PER_INST]
                    keep = waits[-MAX_WAITS_PER_INST:]
                    for j in range(0, len(extra), MAX_WAITS_PER_INST):
                        nop = mybir.InstNoOp(
                            name=nc.get_next_instruction_name(), ins=[], outs=[]
                        )
                        nop.engine = inst.engine
                        nop.sync_info = bass_rust.SyncInfo(
                            on_wait=extra[j : j + MAX_WAITS_PER_INST], on_update=[]
                        )
                        nc.register_instruction(nop, overwrite=True)
                        new_insts.append(nop)
                    si.on_wait = keep
                    inst.sync_info = si
                new_insts.append(inst)
            bb.instructions = new_insts
    return n_split


def _build_nc():
    from contextlib import ExitStack

    import concourse.bass as bass
    import concourse.mybir as mybir
    import concourse.tile as tile
    from concourse.masks import make_identity

    _patch_tile_drain()

    f32 = mybir.dt.float32
    bf16 = mybir.dt.bfloat16

    nc = bass.Bass()
    x_in = nc.declare_dram_parameter("x", [PPC, B, I], f32, isOutput=False)
    w_in = nc.declare_dram_parameter("w", [2, PPC, I, O], f32, isOutput=False)
    out_ext = nc.declare_dram_parameter("out", [PPC, B, O], f32, isOutput=True)

    with ExitStack() as ctx:
        tc = ctx.enter_context(tile.TileContext(nc))
        const_pool = ctx.enter_context(tc.tile_pool(name="const", bufs=1))
        w_pool = ctx.enter_context(tc.tile_pool(name="w", bufs=4))
        d_pool = ctx.enter_context(tc.tile_pool(name="d", bufs=2))
        s_pool = ctx.enter_context(tc.tile_pool(name="s", bufs=2))
        x_pool = ctx.enter_context(tc.tile_pool(name="xp", bufs=2))
        xt_pool = ctx.enter_context(tc.tile_pool(name="xt", bufs=2))
        out_pool = ctx.enter_context(tc.tile_pool(name="op", bufs=2))
        psum_mm = ctx.enter_context(tc.tile_pool(name="pmm", bufs=7, space="PSUM"))
        psum_tr = ctx.enter_context(tc.tile_pool(name="ptr", bufs=1, space="PSUM"))

        ident = const_pool.tile([128, 128], f32)
        make_identity(nc, ident[:])
        ones_bf = const_pool.tile([128, 128], bf16)
        nc.gpsimd.memset(ones_bf[:], 1.0)

        for p in range(PPC):
            # ---- x: one 2MB load, then PE-transpose into bf16 xT tiles
            x_p = x_pool.tile([128, NBT, I], f32, name=f"x_{p}", tag="x")
            nc.sync.dma_start(
                x_p[:], x_in[p].rearrange("(bt q) i -> q bt i", q=128)
            )
            xT_p = xt_pool.tile([128, NBT * I], bf16, name=f"xT_{p}", tag="xT")
            for bt in range(NBT):
                for g in range(2):
                    ptr = psum_tr.tile(
                        [128, 512], f32, name=f"ptr_{p}_{bt}_{g}", tag="tr"
                    )
                    for c in range(4):
                        it = g * 4 + c
                        nc.tensor.transpose(
                            ptr[:, c * 128 : (c + 1) * 128],
                            x_p[:, bt, it * 128 : (it + 1) * 128],
                            ident[:],
                        )
                    nc.scalar.copy(
                        xT_p[:, bt * I + g * 512 : bt * I + (g + 1) * 512], ptr[:]
                    )

            out_sbs = [
                out_pool.tile([128, 2, O], f32, name=f"out_{p}_{jb}", tag="out")
                for jb in range(NBT // 2)
            ]

            # ---- weights: it-pair loads (1MB, 4KB descriptors) on the SP
            #      ring; D = w0 - w1 (bf16) and pair colsums of w1 on DVE
            d_p = d_pool.tile([128, NIT * O], bf16, name=f"d_{p}", tag="d")
            s4 = [
                s_pool.tile([128, O], f32, name=f"s4_{p}_{j}", tag="s4", bufs=4)
                for j in range(NIT // 2)
            ]
            s_bf = s_pool.tile([128, O], bf16, name=f"sbf_{p}", tag="sbf")
            for j in range(NIT // 2):
                w0_t = w_pool.tile([128, 2, O], f32, name=f"w0_{p}_{j}", tag="w0")
                w1_t = w_pool.tile([128, 2, O], f32, name=f"w1_{p}_{j}", tag="w1")
                sl = slice(j * 256, (j + 1) * 256)
                nc.sync.dma_start(
                    w0_t[:], w_in[0, p, sl, :].rearrange("(a q) o -> q a o", a=2)
                )
                nc.sync.dma_start(
                    w1_t[:], w_in[1, p, sl, :].rearrange("(a q) o -> q a o", a=2)
                )
                nc.vector.tensor_sub(
                    d_p[:, j * 2 * O : (j + 1) * 2 * O],
                    w0_t[:].rearrange("q a o -> q (a o)"),
                    w1_t[:].rearrange("q a o -> q (a o)"),
                )
                nc.vector.tensor_add(s4[j][:], w1_t[:, 0, :], w1_t[:, 1, :])
            s01 = s_pool.tile([128, O], f32, name=f"s01_{p}", tag="s01", bufs=1)
            s23 = s_pool.tile([128, O], f32, name=f"s23_{p}", tag="s23", bufs=1)
            nc.vector.tensor_add(s01[:], s4[0][:], s4[1][:])
            nc.vector.tensor_add(s23[:], s4[2][:], s4[3][:])
            nc.vector.tensor_add(s_bf[:], s01[:], s23[:])

            # ---- all 8 (bt, oh) groups of p, pair-major so each group
            #      accumulates as its weight pair lands; bias matmul last
            groups = [(bt, oh) for bt in range(NBT) for oh in range(NOH)]
            pmms = {
                g: psum_mm.tile([128, 512], f32, name=f"pmm_{p}_{g[0]}_{g[1]}", tag="g")
                for g in groups
            }
            for j in range(NIT // 2):
                for bt, oh in groups:
                    for a in range(2):
                        it = 2 * j + a
                        nc.tensor.matmul(
                            pmms[(bt, oh)][:],
                            xT_p[:, bt * I + it * 128 : bt * I + (it + 1) * 128],
                            d_p[:, it * O + oh * 512 : it * O + oh * 512 + 512],
                            start=(it == 0),
                            stop=False,
                        )
            for bt, oh in groups:
                nc.tensor.matmul(
                    pmms[(bt, oh)][:],
                    ones_bf[:],
                    s_bf[:, oh * 512 : (oh + 1) * 512],
                    start=False,
                    stop=True,
                )
                nc.scalar.copy(
                    out_sbs[bt // 2][:, bt % 2, oh * 512 : (oh + 1) * 512],
                    pmms[(bt, oh)][:],
                )
            for jb in range(NBT // 2):
                nc.scalar.dma_start(
                    out_ext[p, jb * 256 : (jb + 1) * 256, :].rearrange(
                        "(c q) o -> q c o", c=2
                    ),
                    out_sbs[jb][:],
                )

    _split_excess_waits(nc)
    return nc


def get_nc():
    if "nc" not in _cache:
        _cache["nc"] = _build_nc()
    return _cache["nc"]


def run(x, w, trace=False, **kwargs):
    from concourse.bass_utils import run_bass_kernel_spmd

    x = np.ascontiguousarray(np.asarray(x, dtype=np.float32))
    w = np.ascontiguousarray(np.asarray(w, dtype=np.float32))
    assert x.shape == (P, B, I) and w.shape == (2, P, I, O)

    nc = get_nc()
    in_maps = [
        {
            "x": np.ascontiguousarray(x[c * PPC : (c + 1) * PPC]),
            "w": np.ascontiguousarray(w[:, c * PPC : (c + 1) * PPC]),
        }
        for c in range(NCORES)
    ]
    res = run_bass_kernel_spmd(nc, in_maps, list(range(NCORES)), trace=trace, **kwargs)
    out = np.concatenate([res.results[c]["out"] for c in range(NCORES)], axis=0)
    return out.astype(np.float32, copy=False), res


def kernel(x, w):
    out, _ = run(x, w, trace=False)
    return out


# revision 12
# speedup vs baseline: 1.1348x; 1.1348x over previous
"""Trainium2 Bass kernel for nn_EvoBinarizedLayer.

Math: out[p,b,o] = sum_i x[p,b,i]*w[0,p,i,o] + (1-x[p,b,i])*w[1,p,i,o]
                 = (x @ D)[p,b,o] + colsum(W1)[p,o],   D = W0 - W1

All inputs are {0,1}-valued f32, so D is {-1,0,1} and both x and D are
exactly representable in bf16; PSUM accumulates in fp32, so the bf16
matmul path is bit-exact. colsum(W1) enters each PSUM accumulation
group as one extra matmul: ones[128,128].T @ s_bf, where s_bf[k,o] =
sum_{it} w1[it*128+k, o] (values <= 8, bf16-exact) is accumulated on DVE.

Sharding: population dim P=32 split across 8 cores (4 each), no
cross-core communication.

DMA routing: weight loads on the SP HWDGE ring, x loads on the gpsimd
SWDGE queue, output stores on the ACT HWDGE ring - three independent
rings so store-completion stalls never block loads.
"""

import numpy as np

P, B, I, O = 32, 512, 1024, 1024
NCORES = 8
PPC = P // NCORES  # populations per core
NIT = I // 128     # i-tiles (contraction)
NBT = B // 128     # b-tiles
NOH = O // 512     # o-halves (PSUM bank width)

_cache = {}

MAX_WAITS_PER_INST = 1


def _patch_tile_drain():
    """This container's walrus caps sem-waits per TPB_CTRL instruction below
    what Tile's final drain needs; spread the waits across nop instructions."""
    import concourse.tile as tile
    import bass_rust
    from concourse.vector_clock import ScopedClock

    if getattr(tile.TileContext, "_drain_patched", False):
        return

    def _drain_and_barrier(self, tick_clock, wait_clock):
        nc = self.nc
        drain_inst = nc.sync.drain()
        wait_clock.add_sem_waits(
            drain_inst.ins, ScopedClock({None: tick_clock.global_clock})
        )
        si = drain_inst.ins.sync_info
        waits = list(si.on_wait or [])
        if len(waits) > 1:
            si.on_wait = waits[:1]
            drain_inst.ins.sync_info = si
            for i in range(1, len(waits)):
                nop = nc.sync.nop()
                nop.ins.sync_info = bass_rust.SyncInfo(
                    on_wait=[waits[i]], on_update=[]
                )
        nc.all_engine_barrier()
        assert self.sems is not None
        popped = nc._tile_sem_poison_stack.pop()
        assert popped is self._sem_poison
        nc.clear_and_free_semaphores(list(self.sems.allocated().values()))
        nc.all_engine_barrier()

    tile.TileContext._drain_and_barrier = _drain_and_barrier
    tile.TileContext._drain_patched = True


def _split_excess_waits(nc):
    """This container's walrus rejects instructions carrying more than a
    couple of sem-waits; hoist excess waits onto same-engine nops placed
    just before the instruction."""
    import concourse.mybir as mybir
    import bass_rust

    n_split = 0
    for fn in nc.m.functions:
        for bb in fn.blocks:
            new_insts = []
            for inst in bb.instructions:
                si = inst.sync_info
                waits = list(si.on_wait) if si and si.on_wait else []
                if len(waits) > MAX_WAITS_PER_INST:
                    n_split += 1
                    extra = waits[: -MAX_WAITS_PER_INST]
                    keep = waits[-MAX_WAITS_PER_INST:]
                    for j in range(0, len(extra), MAX_WAITS_PER_INST):
                        nop = mybir.InstNoOp(
                            name=nc.get_next_instruction_name(), ins=[], outs=[]
                        )
                        nop.engine = inst.engine
                        nop.sync_info = bass_rust.SyncInfo(
                            on_wait=extra[j : j + MAX_WAITS_PER_INST], on_update=[]
                        )
                        nc.register_instruction(nop, overwrite=True)
                        new_insts.append(nop)
                    si.on_wait = keep
                    inst.sync_info = si
                new_insts.append(inst)
            bb.instructions = new_insts
    return n_split


def _build_nc():
    from contextlib import ExitStack

    import concourse.bass as bass
    import concourse.mybir as mybir
    import concourse.tile as tile
    from concourse.masks import make_identity

    _patch_tile_drain()

    f32 = mybir.dt.float32
    bf16 = mybir.dt.bfloat16

    nc = bass.Bass()
    x_in = nc.declare_dram_parameter("x", [PPC, B, I], f32, isOutput=False)
    w_in = nc.declare_dram_parameter("w", [2, PPC, I, O], f32, isOutput=False)
    out_ext = nc.declare_dram_parameter("out", [PPC, B, O], f32, isOutput=True)

    with ExitStack() as ctx:
        tc = ctx.enter_context(tile.TileContext(nc))
        const_pool = ctx.enter_context(tc.tile_pool(name="const", bufs=1))
        w_pool = ctx.enter_context(tc.tile_pool(name="w", bufs=4))
        d_pool = ctx.enter_context(tc.tile_pool(name="d", bufs=2))
        s_pool = ctx.enter_context(tc.tile_pool(name="s", bufs=2))
        x_pool = ctx.enter_context(tc.tile_pool(name="xp", bufs=2))
        xt_pool = ctx.enter_context(tc.tile_pool(name="xt", bufs=2))
        out_pool = ctx.enter_context(tc.tile_pool(name="op", bufs=2))
        psum_mm = ctx.enter_context(tc.tile_pool(name="pmm", bufs=8, space="PSUM"))

        ident = const_pool.tile([128, 128], f32)
        make_identity(nc, ident[:])
        ones_bf = const_pool.tile([128, 128], bf16)
        nc.gpsimd.memset(ones_bf[:], 1.0)

        for p in range(PPC):
            # ---- x: one 2MB load, then PE-transpose into bf16 xT tiles
            x_p = x_pool.tile([128, NBT, I], f32, name=f"x_{p}", tag="x")
            nc.sync.dma_start(
                x_p[:], x_in[p].rearrange("(bt q) i -> q bt i", q=128)
            )
            xT_p = xt_pool.tile([128, NBT * I], bf16, name=f"xT_{p}", tag="xT")
            for bt in range(NBT):
                for g in range(2):
                    ptr = psum_mm.tile(
                        [128, 512], f32, name=f"ptr_{p}_{bt}_{g}", tag="g"
                    )
                    for c in range(4):
                        it = g * 4 + c
                        nc.tensor.transpose(
                            ptr[:, c * 128 : (c + 1) * 128],
                            x_p[:, bt, it * 128 : (it + 1) * 128],
                            ident[:],
                        )
                    nc.scalar.copy(
                        xT_p[:, bt * I + g * 512 : bt * I + (g + 1) * 512], ptr[:]
                    )

            out_sbs = [
                out_pool.tile([128, 2, O], f32, name=f"out_{p}_{jb}", tag="out")
                for jb in range(NBT // 2)
            ]

            # ---- weights: it-pair loads (1MB, 4KB descriptors) on the SP
            #      ring; D = w0 - w1 (bf16) and pair colsums of w1 on DVE
            d_p = d_pool.tile([128, NIT * O], bf16, name=f"d_{p}", tag="d")
            s4 = [
                s_pool.tile([128, O], f32, name=f"s4_{p}_{j}", tag="s4", bufs=4)
                for j in range(NIT // 2)
            ]
            s_bf = s_pool.tile([128, O], bf16, name=f"sbf_{p}", tag="sbf")
            for j in range(NIT // 2):
                w0_t = w_pool.tile([128, 2, O], f32, name=f"w0_{p}_{j}", tag="w0")
                w1_t = w_pool.tile([128, 2, O], f32, name=f"w1_{p}_{j}", tag="w1")
                sl = slice(j * 256, (j + 1) * 256)
                nc.sync.dma_start(
                    w0_t[:], w_in[0, p, sl, :].rearrange("(a q) o -> q a o", a=2)
                )
                nc.sync.dma_start(
                    w1_t[:], w_in[1, p, sl, :].rearrange("(a q) o -> q a o", a=2)
                )
                nc.vector.tensor_sub(
                    d_p[:, j * 2 * O : (j + 1) * 2 * O],
                    w0_t[:].rearrange("q a o -> q (a o)"),
                    w1_t[:].rearrange("q a o -> q (a o)"),
                )
                nc.vector.tensor_add(s4[j][:], w1_t[:, 0, :], w1_t[:, 1, :])
            s01 = s_pool.tile([128, O], f32, name=f"s01_{p}", tag="s01", bufs=1)
            s23 = s_pool.tile([128, O], f32, name=f"s23_{p}", tag="s23", bufs=1)
            nc.vector.tensor_add(s01[:], s4[0][:], s4[1][:])
            nc.vector.tensor_add(s23[:], s4[2][:], s4[3][:])
            nc.vector.tensor_add(s_bf[:], s01[:], s23[:])

            # ---- all 8 (bt, oh) groups of p, pair-major so each group
            #      accumulates as its weight pair lands; bias matmul last
            groups = [(bt, oh) for bt in range(NBT) for oh in range(NOH)]
            pmms = {
                g: psum_mm.tile([128, 512], f32, name=f"pmm_{p}_{g[0]}_{g[1]}", tag="g")
                for g in groups
            }
            for j in range(NIT // 2):
                for bt, oh in groups:
                    for a in range(2):
                        it = 2 * j + a
                        nc.tensor.matmul(
                            pmms[(bt, oh)][:],
                            xT_p[:, bt * I + it * 128 : bt * I + (it + 1) * 128],
                            d_p[:, it * O + oh * 512 : it * O + oh * 512 + 512],
                            start=(it == 0),
                            stop=False,
                        )
            for bt, oh in groups:
                nc.tensor.matmul(
                    pmms[(bt, oh)][:],
                    ones_bf[:],
                    s_bf[:, oh * 512 : (oh + 1) * 512],
                    start=False,
                    stop=True,
                )
                nc.scalar.copy(
                    out_sbs[bt // 2][:, bt % 2, oh * 512 : (oh + 1) * 512],
                    pmms[(bt, oh)][:],
                )
            for jb in range(NBT // 2):
                nc.scalar.dma_start(
                    out_ext[p, jb * 256 : (jb + 1) * 256, :].rearrange(
                        "(c q) o -> q c o", c=2
                    ),
                    out_sbs[jb][:],
                )

    _split_excess_waits(nc)
    return nc


def get_nc():
    if "nc" not in _cache:
        _cache["nc"] = _build_nc()
    return _cache["nc"]


def run(x, w, trace=False, **kwargs):
    from concourse.bass_utils import run_bass_kernel_spmd

    x = np.ascontiguousarray(np.asarray(x, dtype=np.float32))
    w = np.ascontiguousarray(np.asarray(w, dtype=np.float32))
    assert x.shape == (P, B, I) and w.shape == (2, P, I, O)

    nc = get_nc()
    in_maps = [
        {
            "x": np.ascontiguousarray(x[c * PPC : (c + 1) * PPC]),
            "w": np.ascontiguousarray(w[:, c * PPC : (c + 1) * PPC]),
        }
        for c in range(NCORES)
    ]
    res = run_bass_kernel_spmd(nc, in_maps, list(range(NCORES)), trace=trace, **kwargs)
    out = np.concatenate([res.results[c]["out"] for c in range(NCORES)], axis=0)
    return out.astype(np.float32, copy=False), res


def kernel(x, w):
    out, _ = run(x, w, trace=False)
    return out
